# revision 12
# baseline (speedup 1.0000x reference)
"""Trainium2 Bass kernel for nn_EqvSelfAttention (B=4, N=1024, D=256, H=8).

Sharding: data-parallel over (batch b, query-half) -> 8 cores.
Each core computes all 8 heads for its 512 query rows against all 1024 keys.

Math notes (vs reference):
  * 1/sqrt(D)=1/16 folded into Wq (exact power of two).
  * Per-head location-bias MLP: loc_h = sum_d wg2[h,d]*relu(hid_hd) + bg2[h].
    - |wg2| folded into layer-1 weights/bias => z'_hd; sign applied in the
      PE "reduce" matmul that accumulates loc directly onto the content
      logits in PSUM (transposed layout [key, query]).
    - bg2 dropped: constant across keys => softmax-invariant.
  * Softmax computed without max subtraction (logits are O(+-6), exp is safe
    in fp32). Key presence mask folded into V'' = [pk*V | pk]; the 33rd
    column of the A@V'' matmul yields the softmax denominator Z.
  * Absent queries (pq=0) produce uniform attention over ALL keys in the
    reference => Oh = mean(V). Handled by blending with mean(V) after.
"""

import sys
import numpy as np

sys.path.insert(0, "/opt/trn_rl_repo")

B, N, D, H, DH = 4, 1024, 256, 8, 32
R = 512  # query rows per core
NCORES = 8

_CACHE = {}


def _build_program():
    from contextlib import ExitStack

    from concourse import bass, mybir
    import concourse.tile as tile
    from concourse.masks import make_identity

    f32 = mybir.dt.float32
    AF = mybir.ActivationFunctionType
    OP = mybir.AluOpType
    ds = bass.ds

    nc = bass.Bass("TRN2", target_bir_lowering=False, debug=False)

    # ---- I/O declarations (order matters for the PJRT call) ----
    d_y = nc.declare_dram_parameter("y", [N, D], f32, isOutput=False)
    d_yq = nc.declare_dram_parameter("yq", [R, D], f32, isOutput=False)
    d_xp = nc.declare_dram_parameter("xp", [R, 3 * N], f32, isOutput=False)
    d_pkc = nc.declare_dram_parameter("pkc", [128, 8], f32, isOutput=False)
    d_pqr = nc.declare_dram_parameter("pqr", [1, R], f32, isOutput=False)
    d_pqcr = nc.declare_dram_parameter("pqcr", [1, R], f32, isOutput=False)
    d_wq = nc.declare_dram_parameter("wq", [D, D], f32, isOutput=False)
    d_wk = nc.declare_dram_parameter("wk", [D, D], f32, isOutput=False)
    d_wv = nc.declare_dram_parameter("wv", [D, D], f32, isOutput=False)
    d_wo = nc.declare_dram_parameter("wo", [D, D], f32, isOutput=False)
    d_bq = nc.declare_dram_parameter("bq", [1, D], f32, isOutput=False)
    d_bk = nc.declare_dram_parameter("bk", [1, D], f32, isOutput=False)
    d_bv = nc.declare_dram_parameter("bv", [1, D], f32, isOutput=False)
    d_bo = nc.declare_dram_parameter("bo", [1, D], f32, isOutput=False)
    d_bd = nc.declare_dram_parameter("bd", [H, 96, 128], f32, isOutput=False)
    d_rb = nc.declare_dram_parameter("rb", [128, H], f32, isOutput=False)
    d_lr = nc.declare_dram_parameter("lr", [H, 4, 128, 128], f32, isOutput=False)
    d_o = nc.declare_dram_parameter("o", [R, D], f32, isOutput=True)

    with tile.TileContext(nc) as tc:
        with ExitStack() as ctx:
            consts = ctx.enter_context(tc.tile_pool(name="consts", bufs=1))
            persist = ctx.enter_context(tc.tile_pool(name="persist", bufs=1))

            # ---------- constants ----------
            ident = consts.tile([128, 128], f32)
            make_identity(nc, ident)
            ones512 = consts.tile([1, 512], f32)
            nc.vector.memset(ones512, 1.0)
            ones128r = consts.tile([1, 128], f32)
            nc.vector.memset(ones128r, 1.0)
            inv1024c = consts.tile([128, 1], f32)
            nc.vector.memset(inv1024c, 1.0 / 1024.0)

            wqs = consts.tile([128, 2, D], f32)
            nc.sync.dma_start(wqs, d_wq[:, :].rearrange("(t p) d -> p t d", p=128))
            wks = consts.tile([128, 2, D], f32)
            nc.sync.dma_start(wks, d_wk[:, :].rearrange("(t p) d -> p t d", p=128))
            wvs = consts.tile([128, 2, D], f32)
            nc.sync.dma_start(wvs, d_wv[:, :].rearrange("(t p) d -> p t d", p=128))
            wos = consts.tile([128, 2, D], f32)
            nc.sync.dma_start(wos, d_wo[:, :].rearrange("(t p) d -> p t d", p=128))
            bqs = consts.tile([1, D], f32)
            nc.sync.dma_start(bqs, d_bq[:, :])
            bks = consts.tile([1, D], f32)
            nc.sync.dma_start(bks, d_bk[:, :])
            bvs = consts.tile([1, D], f32)
            nc.sync.dma_start(bvs, d_bv[:, :])
            bos = consts.tile([1, D], f32)
            nc.sync.dma_start(bos, d_bo[:, :])
            bdsb = consts.tile([96, H, 128], f32)
            nc.sync.dma_start(bdsb, d_bd[:, :, :].rearrange("h p m -> p h m"))
            rbsb = consts.tile([128, H], f32)
            nc.sync.dma_start(rbsb, d_rb[:, :])
            lrsb = consts.tile([128, H, 4, 128], f32)
            nc.sync.dma_start(lrsb, d_lr[:, :, :, :].rearrange("h c p m -> p h c m"))
            pkcs = consts.tile([128, 8], f32)
            nc.sync.dma_start(pkcs, d_pkc[:, :])
            pqs = consts.tile([1, R], f32)
            nc.sync.dma_start(pqs, d_pqr[:, :])
            pqcs = consts.tile([1, R], f32)
            nc.sync.dma_start(pqcs, d_pqcr[:, :])

            # ---------- persistent activations ----------
            ktsb = persist.tile([128, 2, N], f32)     # K^T [dout, key]
            qtz = persist.tile([128, H, 512], f32)    # per-head zero-padded Q^T
            vsb = persist.tile([128, 8, D], f32)      # V [key, dout]
            v2sb = persist.tile([128, 8, H, 33], f32)  # [pk*V_h | pk]
            vtsb = persist.tile([128, 2, R], f32)     # V^T of my rows
            mvt = persist.tile([128, 2], f32)         # mean_k V  (transposed col)
            xtall = persist.tile([128, 8, 4, 512], f32)  # Xp^T (96 rows used)
            otsb = persist.tile([128, 2, R], f32)     # O^T accumulator
            pqcb = persist.tile([128, R], f32)        # (1-pq) replicated rows

            nc.gpsimd.memset(qtz, 0.0)

            # ---------- phase A: Y^T and projections ----------
            with tc.tile_pool(name="ph_a", bufs=1) as pha, \
                 tc.tile_pool(name="ps_a", bufs=2, space="PSUM") as psa:
                ysb = pha.tile([128, 8, D], f32)
                nc.sync.dma_start(ysb, d_y[:, :].rearrange("(t p) d -> p t d", p=128))
                ysq = pha.tile([128, 4, D], f32)
                nc.sync.dma_start(ysq, d_yq[:, :].rearrange("(t p) d -> p t d", p=128))

                yt = pha.tile([128, 2, N], f32)   # Y^T full batch
                ytq = pha.tile([128, 2, R], f32)  # Y^T my rows
                for dt_ in range(2):
                    for g in range(2):  # groups of 4 n-tiles
                        ps = psa.tile([128, 512], f32)
                        for j in range(4):
                            nt = g * 4 + j
                            nc.tensor.transpose(
                                ps[:, ds(128 * j, 128)],
                                ysb[:, nt, ds(128 * dt_, 128)],
                                ident,
                            )
                        nc.vector.tensor_copy(yt[:, dt_, ds(512 * g, 512)], ps)
                    ps = psa.tile([128, 512], f32)
                    for j in range(4):
                        nc.tensor.transpose(
                            ps[:, ds(128 * j, 128)],
                            ysq[:, j, ds(128 * dt_, 128)],
                            ident,
                        )
                    nc.vector.tensor_copy(ytq[:, dt_], ps)

                qtsb = pha.tile([128, 2, R], f32)
                # Q^T (scaled Wq), K^T, V, V^T projections
                for dt_ in range(2):
                    ps = psa.tile([128, 512], f32)
                    for k_ in range(2):
                        nc.tensor.matmul(
                            ps, wqs[:, k_, ds(128 * dt_, 128)], ytq[:, k_],
                            start=(k_ == 0), stop=False,
                        )
                    nc.tensor.matmul(
                        ps, bqs[0:1, ds(128 * dt_, 128)], ones512,
                        start=False, stop=True,
                    )
                    nc.vector.tensor_copy(qtsb[:, dt_], ps)

                    for half in range(2):
                        ps = psa.tile([128, 512], f32)
                        for k_ in range(2):
                            nc.tensor.matmul(
                                ps, wks[:, k_, ds(128 * dt_, 128)],
                                yt[:, k_, ds(512 * half, 512)],
                                start=(k_ == 0), stop=False,
                            )
                        nc.tensor.matmul(
                            ps, bks[0:1, ds(128 * dt_, 128)], ones512,
                            start=False, stop=True,
                        )
                        nc.vector.tensor_copy(ktsb[:, dt_, ds(512 * half, 512)], ps)

                    ps = psa.tile([128, 512], f32)
                    for k_ in range(2):
                        nc.tensor.matmul(
                            ps, wvs[:, k_, ds(128 * dt_, 128)], ytq[:, k_],
                            start=(k_ == 0), stop=False,
                        )
                    nc.tensor.matmul(
                        ps, bvs[0:1, ds(128 * dt_, 128)], ones512,
                        start=False, stop=True,
                    )
                    nc.vector.tensor_copy(vtsb[:, dt_], ps)

                for nt in range(8):
                    ps = psa.tile([128, 256], f32)
                    for k_ in range(2):
                        nc.tensor.matmul(
                            ps, yt[:, k_, ds(128 * nt, 128)], wvs[:, k_],
                            start=(k_ == 0), stop=False,
                        )
                    nc.tensor.matmul(ps, ones128r, bvs, start=False, stop=True)
                    nc.vector.tensor_copy(vsb[:, nt], ps)

                # per-head zero-padded Q^T slices (keeps content matmuls K=128)
                for h in range(H):
                    base = 32 * (h % 4)
                    nc.vector.tensor_copy(
                        qtz[ds(base, 32), h], qtsb[ds(base, 32), h // 4]
                    )

                # V'' = [pk * V_h | pk]
                for nt in range(8):
                    nc.vector.tensor_scalar(
                        v2sb[:, nt, :, 0:32],
                        vsb[:, nt].rearrange("p (h d) -> p h d", h=H),
                        pkcs[:, nt : nt + 1],
                        None,
                        op0=OP.mult,
                    )
                    nc.vector.tensor_copy(
                        v2sb[:, nt, :, 32:33],
                        pkcs[:, nt : nt + 1].to_broadcast((128, H, 1)),
                    )

                # mean_k V (transposed): mvt[d] = sum_n V[n, d] / 1024
                psmv = psa.tile([128, 2], f32)
                for dt_ in range(2):
                    for nt in range(8):
                        nc.tensor.matmul(
                            psmv[:, dt_ : dt_ + 1],
                            vsb[:, nt, ds(128 * dt_, 128)],
                            inv1024c,
                            start=(nt == 0), stop=(nt == 7),
                        )
                nc.vector.tensor_copy(mvt, psmv)

            # ---------- phase B0: transpose X_pairs ----------
            with tc.tile_pool(name="xp_in", bufs=2) as xpin, \
                 tc.tile_pool(name="ps_t", bufs=2, space="PSUM") as pst:
                for kt in range(8):
                    xt_in = xpin.tile([128, 4, 384], f32)
                    for qt in range(4):
                        nc.sync.dma_start(
                            xt_in[:, qt],
                            d_xp[ds(128 * qt, 128), ds(384 * kt, 384)],
                        )
                    for c4 in range(4):
                        ps = pst.tile([128, 512], f32)
                        for qt in range(4):
                            nc.tensor.transpose(
                                ps[0:96, ds(128 * qt, 128)],
                                xt_in[:, qt, ds(96 * c4, 96)],
                                ident,
                            )
                        if c4 % 2 == 0:
                            nc.scalar.copy(xtall[0:96, kt, c4], ps[0:96])
                        else:
                            nc.vector.tensor_copy(xtall[0:96, kt, c4], ps[0:96])

            # ---------- phase B1: attention main loop ----------
            with tc.tile_pool(name="ps_ct", bufs=2, space="PSUM") as psct, \
                 tc.tile_pool(name="ps_z", bufs=3, space="PSUM") as psz, \
                 tc.tile_pool(name="ps_av", bufs=2, space="PSUM") as psav, \
                 tc.tile_pool(name="ps_bc", bufs=1, space="PSUM") as psbc, \
                 tc.tile_pool(name="rz_p", bufs=3) as rzp, \
                 tc.tile_pool(name="et_p", bufs=2) as etp, \
                 tc.tile_pool(name="fin_p", bufs=2) as finp:
                # replicate (1-pq) across partitions via a K=1 outer product
                psb = psbc.tile([128, 512], f32, name="psbc", tag="bc")
                nc.tensor.matmul(psb, ones128r, pqcs, start=True, stop=True)
                nc.vector.tensor_copy(pqcb, psb)
                for h in range(H):
                    av = psav.tile([128, 512], f32)
                    for kt in range(8):
                        ct = psct.tile([128, 512], f32)
                        nc.tensor.matmul(
                            ct,
                            ktsb[:, h // 4, ds(128 * kt, 128)],
                            qtz[:, h],
                            start=True, stop=False,
                        )
                        for c4 in range(4):
                            zps = psz.tile([128, 512], f32)
                            nc.tensor.matmul(
                                zps, bdsb[:, h], xtall[0:96, kt, c4],
                                start=True, stop=True,
                            )
                            rz = rzp.tile([128, 512], f32)
                            if c4 % 2 == 0:
                                nc.scalar.activation(
                                    rz, zps, AF.Relu, bias=rbsb[:, h : h + 1]
                                )
                            else:
                                nc.vector.tensor_scalar(
                                    rz, zps, rbsb[:, h : h + 1], 0.0,
                                    op0=OP.add, op1=OP.max,
                                )
                            nc.tensor.matmul(
                                ct, lrsb[:, h, c4], rz,
                                start=False, stop=(c4 == 3),
                            )
                        et = etp.tile([128, 512], f32)
                        nc.scalar.activation(et, ct, AF.Exp)
                        nc.tensor.matmul(
                            av[0:33], v2sb[:, kt, h], et,
                            start=(kt == 0), stop=(kt == 7),
                        )
                    # finalize head h
                    rec = finp.tile([1, 512], f32)
                    nc.vector.reciprocal(rec, av[32:33])
                    rpq = finp.tile([1, 512], f32)
                    nc.vector.tensor_mul(rpq, rec, pqs)
                    rpqb = psbc.tile([128, 512], f32, name="rpqb", tag="bc")
                    nc.tensor.matmul(
                        rpqb[0:32], ones128r[0:1, 0:32], rpq, start=True, stop=True
                    )
                    rpqs = finp.tile([32, 512], f32)
                    nc.vector.tensor_copy(rpqs, rpqb[0:32])
                    t2 = finp.tile([32, 512], f32)
                    nc.vector.tensor_mul(t2, av[0:32], rpqs)
                    mv0 = finp.tile([32, 1], f32)
                    nc.vector.tensor_copy(
                        mv0, mvt[ds(32 * (h % 4), 32), h // 4 : h // 4 + 1]
                    )
                    t3 = finp.tile([32, 512], f32)
                    nc.vector.tensor_mul(
                        t3, mv0.to_broadcast((32, 512)), pqcb[0:32]
                    )
                    t4 = finp.tile([32, 512], f32)
                    nc.vector.tensor_add(t4, t2, t3)
                    vt0 = finp.tile([32, 512], f32)
                    nc.vector.tensor_copy(vt0, vtsb[ds(32 * (h % 4), 32), h // 4])
                    t5 = finp.tile([32, 512], f32)
                    nc.vector.tensor_add(t5, t4, vt0)
                    nc.vector.tensor_copy(otsb[ds(32 * (h % 4), 32), h // 4], t5)

            # ---------- phase C: O = O + relu(O @ Wo + bo) ----------
            with tc.tile_pool(name="ps_o", bufs=2, space="PSUM") as pso, \
                 tc.tile_pool(name="o_p", bufs=2) as op_:
                for j in range(4):
                    pso1 = pso.tile([128, 256], f32)
                    for dt_ in range(2):
                        nc.tensor.transpose(
                            pso1[:, ds(128 * dt_, 128)],
                            otsb[:, dt_, ds(128 * j, 128)],
                            ident,
                        )
                    oj = op_.tile([128, 256], f32)
                    nc.vector.tensor_copy(oj, pso1)

                    pso2 = pso.tile([128, 256], f32)
                    for dt_ in range(2):
                        nc.tensor.matmul(
                            pso2, otsb[:, dt_, ds(128 * j, 128)], wos[:, dt_],
                            start=(dt_ == 0), stop=False,
                        )
                    nc.tensor.matmul(pso2, ones128r, bos, start=False, stop=True)
                    r2 = op_.tile([128, 256], f32)
                    nc.scalar.activation(r2, pso2, AF.Relu)
                    ofin = op_.tile([128, 256], f32)
                    nc.vector.tensor_add(ofin, oj, r2)
                    nc.sync.dma_start(d_o[ds(128 * j, 128), :], ofin)

    _split_multiwait(nc, mybir)
    return nc


def _split_multiwait(nc, mybir):
    """This walrus build only encodes ONE sem-wait per instruction; Tile's
    tail drain carries several. Split extras onto preceding NoOps."""
    for f in nc.m.functions:
        for blk in f.blocks:
            insts = list(blk.instructions)
            changed = False
            newlist = []
            for ins in insts:
                si = ins.sync_info
                if si is not None and len(si.on_wait) > 1:
                    waits = list(si.on_wait)
                    for j, w in enumerate(waits[:-1]):
                        newlist.append(
                            mybir.InstNoOp(
                                name=f"{ins.name}_splitw{j}",
                                engine=ins.engine,
                                ins=[],
                                outs=[],
                                sync_info=mybir.SyncInfo(on_wait=[w], on_update=[]),
                            )
                        )
                    ins.sync_info = mybir.SyncInfo(
                        on_wait=[waits[-1]], on_update=list(si.on_update)
                    )
                    changed = True
                newlist.append(ins)
            if changed:
                blk.instructions = newlist


def _host_constants(Wg1, bg1, wg2, bg2):
    """Build the folded block-diag layer-1 weights, relu biases and the
    signed reduce matrices."""
    aw = np.abs(wg2)  # [H, 3]
    sw = np.sign(wg2).astype(np.float32)
    kk = np.arange(32)

    bd = np.zeros((H, 96, 128), np.float32)
    rb = np.zeros((128, H), np.float32)
    lr = np.zeros((H, 4, 128, 128), np.float32)
    for c in range(3):
        for s in range(3):
            # bd[h, 3kk+c, 4kk+s] = |wg2[h,s]| * Wg1[h,c,s]
            bd[:, 3 * kk + c, 4 * kk + s] = aw[:, s : s + 1] * Wg1[:, c, s : s + 1]
    for s in range(3):
        rb[4 * kk + s, :] = (aw[:, s] * bg1[:, s])[np.newaxis, :]
        for c4 in range(4):
            lr[:, c4, 4 * kk + s, 32 * c4 + kk] = sw[:, s : s + 1]
    return bd, rb, lr


def kernel(**inputs):
    from concourse.bass_utils import run_bass_kernel_spmd

    X = {k: np.asarray(v, dtype=np.float32) for k, v in inputs.items()}
    Y = X["Y_lift"]          # [B, N, D]
    XP = X["X_pairs"]        # [B, N, N, 3]
    PQ = X["presence_q"]     # [B, N]
    PK = X["presence_k"]     # [B, N]

    bd, rb, lr = _host_constants(X["Wg1"], X["bg1"], X["wg2"], X["bg2"])

    wq = np.ascontiguousarray(X["Wq"] / 16.0)
    bq = np.ascontiguousarray((X["bq"] / 16.0).reshape(1, D))
    wk, bk = X["Wk"], X["bk"].reshape(1, D)
    wv, bv = X["Wv"], X["bv"].reshape(1, D)
    wo, bo = X["Wo"], X["bo"].reshape(1, D)

    if "nc" not in _CACHE:
        _CACHE["nc"] = _build_program()
    nc = _CACHE["nc"]

    in_maps = []
    for core in range(NCORES):
        b, half = core // 2, core % 2
        rows = slice(half * R, half * R + R)
        in_maps.append(
            {
                "y": np.ascontiguousarray(Y[b]),
                "yq": np.ascontiguousarray(Y[b, rows]),
                "xp": np.ascontiguousarray(XP[b, rows].reshape(R, 3 * N)),
                "pkc": np.ascontiguousarray(PK[b].reshape(8, 128).T),
                "pqr": np.ascontiguousarray(PQ[b, rows].reshape(1, R)),
                "pqcr": np.ascontiguousarray(1.0 - PQ[b, rows].reshape(1, R)),
                "wq": wq,
                "wk": np.ascontiguousarray(wk),
                "wv": np.ascontiguousarray(wv),
                "wo": np.ascontiguousarray(wo),
                "bq": bq,
                "bk": np.ascontiguousarray(bk),
                "bv": np.ascontiguousarray(bv),
                "bo": np.ascontiguousarray(bo),
                "bd": bd,
                "rb": rb,
                "lr": lr,
            }
        )

    res = run_bass_kernel_spmd(nc, in_maps, core_ids=list(range(NCORES)))
    out = np.empty((B, N, D), np.float32)
    for core in range(NCORES):
        b, half = core // 2, core % 2
        out[b, half * R : half * R + R] = res.results[core]["o"]
    return out
